# revision 12
# baseline (speedup 1.0000x reference)
"""LoRA cross-attention kernel for 8 Trainium2 NeuronCores.

Sharding: batch*heads across 8 cores. Core d handles batch b=d//4 and the
4-head slice h in [4*(d%4), 4*(d%4)+4)  (inner slice of 256 = 4*64).
Each core computes q/k/v projections (+LoRA on k,v) for its slice, attention,
and a partial to_out (tensor-parallel over inner). Host sums the 4 partials
per batch and adds the bias.

Device dataflow (all matmuls bf16 operands, fp32 PSUM accumulate):
  xT,cT   [128,8,2048]  x^T / context^T via xbar-transpose DMA loads
  lowT    [32,2048]     [Ak;Av]-low rank projections of context
  qT,kT   [128,2,2048]  q^T, k^T (i on partitions);  kT includes LoRA up-proj
  v       [128,16,4,65] v in [m, head, dh+1] layout, col 64 = ones
  simT    psum[m,2,512] per head pair via row-tiled (tile_position) matmuls
  e       exp(SCALE*simT) on ScalarE -> bf16
  attn@v  lhsT=v_aug[m,65], rhs=e -> psum[65,n]: rows 0:64 out^T, row 64 denom
  norm    recip(denom) broadcast via K=1 matmul, DVE multiply
  to_out  WoT.T @ outT -> partial final^T [1024,2048] fp32 -> HBM
"""

import numpy as np
import ml_dtypes

import concourse.bass as bass
import concourse.mybir as mybir
import concourse.tile as tile

BF16 = mybir.dt.bfloat16
F32 = mybir.dt.float32
AF = mybir.ActivationFunctionType

N = 2048      # query length
M = 2048      # context length
D = 1024      # model dim
IS = 256      # inner slice per core (4 heads * 64)
DH = 64
NHEADS = 4    # heads per core
SCALE = DH ** -0.5
NB = 512      # n-block (free dim tile)
N_NB = N // NB
N_MB = M // 128

_NC_CACHE = {}


def _emit(tc, nc, d):
    from contextlib import ExitStack
    ctx = ExitStack()
    P1 = ctx.enter_context(tc.tile_pool(name="persist", bufs=1))
    WK = ctx.enter_context(tc.tile_pool(name="work", bufs=4))
    PS = ctx.enter_context(tc.tile_pool(name="psum", bufs=2, space="PSUM"))
    PO = ctx.enter_context(tc.tile_pool(name="psum_o", bufs=2, space="PSUM"))

    xT = P1.tile([128, 8, N], BF16)
    cT = P1.tile([128, 8, M], BF16)
    wq = P1.tile([128, 8, IS], BF16)
    wk = P1.tile([128, 8, IS], BF16)
    wv = P1.tile([128, 8, IS], BF16)
    ab = P1.tile([128, 8, 32], BF16)
    bk = P1.tile([32, IS], BF16)
    bv = P1.tile([32, IS], BF16)
    wo = P1.tile([128, 2, D], BF16)
    qT = P1.tile([128, 2, N], BF16)
    kT = P1.tile([128, 2, M], BF16)
    vA = P1.tile([128, N_MB, NHEADS, DH + 1], BF16)
    oT = P1.tile([128, 2, N], BF16)
    low = P1.tile([32, M], BF16)
    ones64 = P1.tile([1, DH], BF16)
    ident = P1.tile([64, 64], BF16)

    # ---- weight / input loads ----
    nc.sync.dma_start(wq[:], d["wqT"].rearrange("(ko ki) i -> ki ko i", ki=128))
    nc.sync.dma_start(wk[:], d["wkT"].rearrange("(ko ki) i -> ki ko i", ki=128))
    nc.sync.dma_start(wv[:], d["wvT"].rearrange("(ko ki) i -> ki ko i", ki=128))
    nc.sync.dma_start(ab[:], d["abT"].rearrange("(ko ki) r -> ki ko r", ki=128))
    nc.sync.dma_start(bk[:], d["bkT0"][:])
    nc.sync.dma_start(bv[:], d["b0vT"][:])
    nc.sync.dma_start(wo[:], d["woT"].rearrange("(ko ki) dd -> ki ko dd", ki=128))
    nc.gpsimd.memset(ones64[:], 1.0)
    nc.gpsimd.memset(vA[:, :, :, DH], 1.0)
    from concourse.masks import make_identity
    make_identity(nc, ident[:])

    for kb in range(8):
        nc.sync.dma_start_transpose(cT[:, kb, :], d["cbf"][:, kb * 128:(kb + 1) * 128])
    for kb in range(8):
        nc.sync.dma_start_transpose(xT[:, kb, :], d["xbf"][:, kb * 128:(kb + 1) * 128])

    # ---- lowT = [Ak|Av]^T-proj of context: [32, M] ----
    for nb in range(M // NB):
        pl = PS.tile([128, 2, NB], F32, tag="ps")
        for kb in range(8):
            nc.tensor.matmul(pl[0:32, 0, :], ab[:, kb, :], cT[:, kb, bass.ts(nb, NB)],
                             start=(kb == 0), stop=(kb == 7))
        nc.vector.tensor_copy(low[:, bass.ts(nb, NB)], pl[0:32, 0, :])

    def proj_qk(ib):
        # qT block ib
        for nb in range(N_NB):
            pq = PS.tile([128, 2, NB], F32, tag="ps")
            for kb in range(8):
                nc.tensor.matmul(pq[:, 0, :], wq[:, kb, bass.ts(ib, 128)],
                                 xT[:, kb, bass.ts(nb, NB)],
                                 start=(kb == 0), stop=(kb == 7))
            nc.vector.tensor_copy(qT[:, ib, bass.ts(nb, NB)], pq[:, 0, :])
        # kT block ib (base + LoRA up-projection)
        for nb in range(M // NB):
            pk = PS.tile([128, 2, NB], F32, tag="ps")
            for kb in range(8):
                nc.tensor.matmul(pk[:, 0, :], wk[:, kb, bass.ts(ib, 128)],
                                 cT[:, kb, bass.ts(nb, NB)],
                                 start=(kb == 0), stop=False)
            nc.tensor.matmul(pk[:, 0, :], bk[:, bass.ts(ib, 128)],
                             low[:, bass.ts(nb, NB)], start=False, stop=True)
            nc.vector.tensor_copy(kT[:, ib, bass.ts(nb, NB)], pk[:, 0, :])

    def proj_v():
        for mb in range(N_MB):
            pv = PS.tile([128, 2, NB], F32, tag="ps")
            for kb in range(8):
                nc.tensor.matmul(pv[:, 0, 0:IS], cT[:, kb, bass.ts(mb, 128)],
                                 wv[:, kb, :], start=(kb == 0), stop=False)
            nc.tensor.matmul(pv[:, 0, 0:IS], low[:, bass.ts(mb, 128)], bv[:],
                             start=False, stop=True)
            nc.vector.tensor_copy(
                vA[:, mb, :, 0:DH],
                pv[:, 0, 0:IS].rearrange("p (h e) -> p h e", h=NHEADS))

    def attention(p):
        for nb in range(N_NB):
            po = PO.tile([128, 2, NB], F32, tag="po")
            for mb in range(N_MB):
                ps = PS.tile([128, 2, NB], F32, tag="ps")
                nc.tensor.matmul(ps[:, 0, :], kT[0:64, p, bass.ts(mb, 128)],
                                 qT[0:64, p, bass.ts(nb, NB)],
                                 start=True, stop=True, tile_position=(0, 0))
                nc.tensor.matmul(ps[:, 1, :], kT[64:128, p, bass.ts(mb, 128)],
                                 qT[64:128, p, bass.ts(nb, NB)],
                                 start=True, stop=True, tile_position=(64, 0))
                e = WK.tile([128, 2, NB], BF16, tag="e")
                nc.scalar.activation(e[:], ps[:], AF.Exp, scale=SCALE)
                for j in range(2):
                    nc.tensor.matmul(po[0:DH + 1, j, :], vA[:, mb, 2 * p + j, :],
                                     e[:, j, :], start=(mb == 0), stop=(mb == N_MB - 1),
                                     skip_group_check=True)
            # normalize: out[dh, n] *= 1/denom[n]
            den = WK.tile([1, 2, NB], BF16, tag="den")
            nc.scalar.copy(den[:], po[DH:DH + 1, :, :])
            bc = PS.tile([128, 2, NB], F32, tag="ps")
            for j in range(2):
                nc.tensor.matmul(bc[0:DH, j, :], ones64[:], den[:, j, :],
                                 start=True, stop=True)
            bcs = WK.tile([64, 2, NB], F32, tag="bcs")
            nc.vector.reciprocal(bcs[:], bc[0:DH, :, :])
            # even head of the pair lands on partitions 0:64 directly
            nc.vector.tensor_mul(out=oT[0:64, p, bass.ts(nb, NB)],
                                 in0=po[0:DH, 0, :], in1=bcs[:, 0, :])
            # odd head: normalize to a temp, shift to partitions 64:128 via
            # identity matmul (col tile_position), copy back aligned
            o4h = WK.tile([64, NB], BF16, tag="o4h")
            nc.vector.tensor_mul(out=o4h[:], in0=po[0:DH, 1, :], in1=bcs[:, 1, :])
            psh = PS.tile([128, 2, NB], F32, tag="ps")
            nc.tensor.matmul(psh[64:128, 0, :], ident[:], o4h[:],
                             start=True, stop=True, tile_position=(0, 64))
            nc.vector.tensor_copy(oT[64:128, p, bass.ts(nb, NB)],
                                  psh[64:128, 0, :])

    def to_out():
        for db in range(8):
            for nb in range(N_NB):
                pf = PS.tile([128, 2, NB], F32, tag="ps")
                for kb in range(2):
                    nc.tensor.matmul(pf[:, 0, :], wo[:, kb, bass.ts(db, 128)],
                                     oT[:, kb, bass.ts(nb, NB)],
                                     start=(kb == 0), stop=(kb == 1))
                f = WK.tile([128, NB], F32, tag="fout")
                nc.any.tensor_copy(f[:], pf[:, 0, :])
                nc.sync.dma_start(
                    d["outT"][bass.ts(db, 128), bass.ts(nb, NB)], f[:])

    proj_qk(0)
    proj_v()
    attention(0)
    proj_qk(1)
    attention(1)
    to_out()

    ctx.close()


def _legalize_mm_waits(nc, cap=2):
    """walrus's MM struct holds at most `cap` sync waits; the Tile scheduler
    occasionally emits more. Move excess waits onto preceding PE instructions
    (same engine, earlier in program order → strictly safe)."""
    for f in nc.m.functions:
        for bb in f.blocks:
            pe_idx = [i for i, ins in enumerate(bb.instructions)
                      if str(getattr(ins, "engine", "")) == "EngineType.PE"]
            for pos, i in enumerate(pe_idx):
                ins = bb.instructions[i]
                if type(ins).__name__ != "InstMatmult":
                    continue
                si = ins.sync_info
                if not si or not si.on_wait or len(si.on_wait) <= cap:
                    continue
                excess = list(si.on_wait[cap:])
                ins.sync_info = type(si)(on_wait=list(si.on_wait[:cap]),
                                         on_update=si.on_update)
                j = pos - 1
                while excess and j >= 0:
                    prev = bb.instructions[pe_idx[j]]
                    psi = prev.sync_info
                    pw = list(psi.on_wait) if (psi and psi.on_wait) else []
                    room = cap - len(pw)
                    if room > 0:
                        take, excess = excess[:room], excess[room:]
                        prev.sync_info = type(si)(
                            on_wait=pw + take,
                            on_update=(psi.on_update if psi else []))
                    j -= 1
                assert not excess, f"could not legalize waits on {ins.name}"


def build_nc():
    from concourse import bacc
    nc = bacc.Bacc(None, target_bir_lowering=False)
    d = {
        "xbf": nc.dram_tensor("xbf", [N, D], BF16, kind="ExternalInput"),
        "cbf": nc.dram_tensor("cbf", [M, D], BF16, kind="ExternalInput"),
        "wqT": nc.dram_tensor("wqT", [D, IS], BF16, kind="ExternalInput"),
        "wkT": nc.dram_tensor("wkT", [D, IS], BF16, kind="ExternalInput"),
        "wvT": nc.dram_tensor("wvT", [D, IS], BF16, kind="ExternalInput"),
        "abT": nc.dram_tensor("abT", [D, 32], BF16, kind="ExternalInput"),
        "bkT0": nc.dram_tensor("bkT0", [32, IS], BF16, kind="ExternalInput"),
        "b0vT": nc.dram_tensor("b0vT", [32, IS], BF16, kind="ExternalInput"),
        "woT": nc.dram_tensor("woT", [IS, D], BF16, kind="ExternalInput"),
        "outT": nc.dram_tensor("outT", [D, N], F32, kind="ExternalOutput"),
    }
    with tile.TileContext(nc) as tc:
        _emit(tc, nc, d)
    nc.compile()
    return nc


def get_nc():
    if "nc" not in _NC_CACHE:
        _NC_CACHE["nc"] = build_nc()
    return _NC_CACHE["nc"]


def make_in_maps(x, context, task_idx, Wq, Wk, Wv, Ak, Bk, Av, Bv, Wo):
    bf = ml_dtypes.bfloat16
    xb = np.ascontiguousarray(x).astype(bf)
    cb = np.ascontiguousarray(context).astype(bf)
    in_maps = []
    for dev in range(8):
        b = dev // 4
        isl = slice(IS * (dev % 4), IS * (dev % 4) + IS)
        t = int(task_idx[b])
        z16 = np.zeros((16, IS), np.float32)
        in_maps.append({
            "xbf": xb[b],
            "cbf": cb[b],
            "wqT": np.ascontiguousarray(Wq[isl].T).astype(bf),
            "wkT": np.ascontiguousarray(Wk[isl].T).astype(bf),
            "wvT": np.ascontiguousarray(Wv[isl].T).astype(bf),
            "abT": np.concatenate([Ak[t].T, Av[t].T], axis=1).astype(bf),
            "bkT0": np.concatenate([Bk[t][isl].T, z16], axis=0).astype(bf),
            "b0vT": np.concatenate([z16, Bv[t][isl].T], axis=0).astype(bf),
            "woT": np.ascontiguousarray(Wo[:, isl].T).astype(bf),
        })
    return in_maps


def combine(results, bo):
    B = 2
    out = np.empty((B, N, D), np.float32)
    for b in range(B):
        acc = results[4 * b]["outT"].astype(np.float32).copy()
        for j in range(1, 4):
            acc += results[4 * b + j]["outT"]
        out[b] = acc.T
    out += bo.astype(np.float32)
    return out


def kernel(x, context, mask, task_idx, Wq, Wk, Wv, Ak, Bk, Av, Bv, Wo, bo,
           _trace=False):
    # mask is all-ones per the input spec; softmax ignores it.
    from concourse.bass_utils import run_bass_kernel_spmd
    args = [np.asarray(a) for a in
            (x, context, task_idx, Wq, Wk, Wv, Ak, Bk, Av, Bv, Wo)]
    in_maps = make_in_maps(*args)
    nc = get_nc()
    res = run_bass_kernel_spmd(nc, in_maps, core_ids=list(range(8)),
                               trace=_trace)
    out = combine(res.results, np.asarray(bo))
    if _trace:
        return out, res
    return out
